# revision 38
# baseline (speedup 1.0000x reference)
"""Trainium2 Bass kernel for nn_AssociatorLoss (low-rank dot formulation).

Reference (B=32, N=32), a = cayley_cube (B,N,N,N):
    one[b,i,j,k,l] = sum_m a[b,i,m,l] * a[b,j,k,m]
    two[b,i,j,k,l] = sum_m a[b,m,k,l] * a[b,i,j,m]
    kl = sum(two * (log(two) - log(one))) / B

Key identity: in (u=(i,j), v=(k,l)) coordinates two = P·Q with
P[u,m] = a[i,j,m], Q[m,v] = a[m,k,l] (rank 32).  Hence for any X in that
layout:  sum(two ⊙ X) = sum_m,v Q[m,v] · (P^T X)[m,v].
So both dot products reduce to small PE matmuls H = P^T X accumulated over
(i,j)-chunks, with X = ln(two) and X = blockT(ln(one)):

per chunk c (128 (ij)-rows x 1024 (kl)-cols), per batch elem:
  PE : op = one-chunk  [p=(i,l), f=(k,j)]  (2 row-packed K=32 matmuls)
       tp = two-chunk  [p=(i,j), f=(k,l)]  (2 row-packed K=32 matmuls)
  ACT: lt = Ln(tp), lo = Ln(op)   (PSUM -> SBUF bf16)
  DVE: lot = blockT32(lo)         -> [p=(i,j), f=(k,l)]
  PE : H4 quadrants += af_c^T @ {lt,lot}-halves  (4 col-packed matmuls,
       K=128, M=32, accumulated over the 8 chunks in PSUM)
per batch elem:
  DVE: acc[:,b] = rowsum( (abq * sgn) ⊙ H4 )   (sgn=+1 for ln-two quads,
       -1 for ln-one quads); host sums acc and divides by B.

Data-parallel over b: 4 batch elems per core, partial sums combined on host.
"""

import sys

for _p in ("/opt/trn_rl_repo",):
    if _p not in sys.path:
        sys.path.insert(0, _p)

import numpy as np

import concourse.bacc as bacc
import concourse.mybir as mybir
import concourse.tile as tile
from concourse.bass_utils import run_bass_kernel_spmd

B, N = 32, 32
N_CORES = 8
B_LOCAL = B // N_CORES  # 4
NCHUNK = (N * N) // 128  # 8 chunks of 128 (ij)-rows per batch element
F32 = mybir.dt.float32
BF16 = mybir.dt.bfloat16


def build(b_local=B_LOCAL):
    nc = bacc.Bacc(None, target_bir_lowering=False)
    a_ext = nc.declare_dram_parameter("cayley_cube", [b_local, N, N, N], F32, isOutput=False)
    out_ext = nc.declare_dram_parameter("out", [128, b_local], F32, isOutput=True)

    av = a_ext.rearrange("b x y z -> b x (y z)")
    # af rows are (ij)-chunk layout: af[p, c*32+m] = a[4c + p//32, p%32, m]
    av4 = a_ext.rearrange("b (c il) j m -> b (il j) c m", c=NCHUNK, il=4)
    # abq halves: row h*32+m holds a[m, (kl) half h]
    av5 = a_ext.rearrange("b m (h k2) l -> b h m (k2 l)", h=2, k2=16)
    # aof[m, (c,il,l)] = a[4c+il, m, l] == ay2[:, 128c:...] (one-matmul lhsT)
    av6 = a_ext.rearrange("b (c il) m l -> b m c il l", c=NCHUNK, il=4)

    mult = mybir.AluOpType.mult
    Ln = mybir.ActivationFunctionType.Ln

    with tile.TileContext(nc) as tc:
        with (
            tc.tile_pool(name="apool", bufs=2) as apool,
            tc.tile_pool(name="spool", bufs=4) as spool,
            tc.tile_pool(name="scratch", bufs=1) as scratch,
            tc.tile_pool(name="psumO", bufs=2, space="PSUM") as psumO,
            tc.tile_pool(name="psumT", bufs=1, space="PSUM") as psumT,
            tc.tile_pool(name="psumH", bufs=2, space="PSUM") as psumH,
        ):
            sgn = scratch.tile([128, 1], F32)
            nc.vector.memset(sgn[0:64, :], 1.0)
            nc.vector.memset(sgn[64:128, :], -1.0)
            acc = scratch.tile([128, b_local], F32)
            junk = scratch.tile([128, 512], BF16)

            def emit_prep_dma(b):
                """Fire all HBM loads for batch b (casting loads on the
                gpsimd queue, plain f32 loads on sync)."""
                t = {}
                # transpose source rides the idle sync queue as plain f32
                # (casting DMAs are gpsimd-only); a DVE copy does the cast
                abf = apool.tile([32, 1024], F32, tag="abf")
                t["abf"] = abf
                nc.sync.dma_start(out=abf[:], in_=av[b])
                ab4 = apool.tile([128, 1024], BF16, tag="ab4")
                t["ab4"] = ab4
                aof2 = apool.tile([64, NCHUNK * 128], BF16, tag="aof2")
                t["aof2"] = aof2
                for q in range(2):
                    nc.gpsimd.dma_start(
                        out=aof2[32 * q:32 * q + 32, :].rearrange(
                            "p (c il l) -> p c il l", c=NCHUNK, il=4, l=N),
                        in_=av6[b],
                    )
                for q in (2, 3):
                    nc.gpsimd.dma_start(out=ab4[32 * q:32 * q + 32, :], in_=av[b])
                af = apool.tile([128, NCHUNK * 32], BF16, tag="af")
                t["af"] = af
                nc.gpsimd.dma_start(
                    out=af[:].rearrange("p (c m) -> p c m", c=NCHUNK, m=N),
                    in_=av4[b],
                )
                abq = apool.tile([128, 512], F32, tag="abq")
                t["abq"] = abq
                for q in range(4):
                    nc.sync.dma_start(out=abq[32 * q:32 * q + 32, :],
                                      in_=av5[b, q % 2])
                return t

            def emit_prep_ops(t, first=False):
                """On-chip transposes for batch b.  at is built as two half
                transposes straight into the partitions each matmul reads,
                so no at replica copy is needed."""
                abf = t["abf"]
                abt = apool.tile([32, 1024], BF16, tag="abt")
                nc.vector.tensor_copy(abt[:], abf[:])
                # att: [0:32, 0:512]=at-h1, [32:64, 512:]=at-h2, [64:96]=at2
                # at[z, y*32+x] = a[x,y,z]; at2[z, x*32+y] = a[x,y,z]
                att = apool.tile([128, 1024], BF16, tag="att")
                t["att"] = att
                nc.vector.transpose(att[0:32, 0:512], abt[:, 0:512])
                nc.vector.transpose(att[32:64, 512:1024], abt[:, 512:1024])
                # at2 from the two at halves (y<16 from h1, y>=16 from h2)
                if first:
                    # x-split: chunk 0 reads only cols 0:128, so emit those
                    # slices (both y-halves + replica) before the rest
                    for x0, x1 in ((0, 4), (4, N)):
                        nc.vector.tensor_copy(
                            att[64:96, 32 * x0:32 * x1].rearrange(
                                "p (x y) -> p y x", x=x1 - x0, y=N)[:, 0:16, :],
                            att[0:32, 0:512].rearrange(
                                "p (y x) -> p y x", y=16, x=N)[:, :, x0:x1],
                        )
                        nc.vector.tensor_copy(
                            att[64:96, 32 * x0:32 * x1].rearrange(
                                "p (x y) -> p y x", x=x1 - x0, y=N)[:, 16:32, :],
                            att[32:64, 512:1024].rearrange(
                                "p (y x) -> p y x", y=16, x=N)[:, :, x0:x1],
                        )
                        nc.vector.tensor_copy(att[96:128, 32 * x0:32 * x1],
                                              att[64:96, 32 * x0:32 * x1])
                else:
                    for pb in (64, 96):
                        nc.gpsimd.tensor_copy(
                            att[pb:pb + 32, :].rearrange(
                                "p (x y) -> p y x", x=N, y=N)[:, 0:16, :],
                            att[0:32, 0:512].rearrange("p (y x) -> p y x", y=16, x=N),
                        )
                        nc.gpsimd.tensor_copy(
                            att[pb:pb + 32, :].rearrange(
                                "p (x y) -> p y x", x=N, y=N)[:, 16:32, :],
                            att[32:64, 512:1024].rearrange("p (y x) -> p y x", y=16, x=N),
                        )

            def emit_h(h4, af, pend):
                """H-matmuls for a finished chunk (emitted one chunk late so
                the in-order PE queue never stalls on the DVE transpose)."""
                lt, lot, cs, st, sp = pend
                nc.tensor.matmul(h4[0:32, :], af[:, cs], lt[:, 0:512],
                                 start=st, stop=sp, tile_position=(0, 0))
                nc.tensor.matmul(h4[32:64, :], af[:, cs], lt[:, 512:1024],
                                 start=st, stop=sp, tile_position=(0, 32))
                nc.tensor.matmul(h4[64:96, :], af[:, cs], lot[:, 0:512],
                                 start=st, stop=sp, tile_position=(0, 64))
                nc.tensor.matmul(h4[96:128, :], af[:, cs], lot[:, 512:1024],
                                 start=st, stop=sp, tile_position=(0, 96))

            prep = emit_prep_dma(0)
            emit_prep_ops(prep, first=True)
            for b in range(b_local):
                ab4, att = prep["ab4"], prep["att"]
                aof2, af, abq = prep["aof2"], prep["af"], prep["abq"]
                h4 = psumH.tile([128, 512], F32, tag="h4")
                pend = None

                for c in range(NCHUNK):
                    if c == 0 and b + 1 < b_local:
                        nprep = emit_prep_dma(b + 1)
                    if c == 3 and b + 1 < b_local:
                        emit_prep_ops(nprep)
                    if c == 6 and b + 1 < b_local:
                        prep = nprep
                    ms = slice(128 * c, 128 * (c + 1))
                    cs = slice(32 * c, 32 * (c + 1))
                    op = psumO.tile([128, 1024], F32, tag="op")
                    tp = psumT.tile([128, 1024], F32, tag="tp")

                    # one: out[p=(i,l), f=(k,j)] = sum_m aof[m,(i,l)] at[m,(k,j)]
                    nc.tensor.matmul(op[:, 0:512], aof2[0:32, ms], att[0:32, 0:512],
                                     start=True, stop=True, tile_position=(0, 0))
                    nc.tensor.matmul(op[:, 512:1024], aof2[32:64, ms], att[32:64, 512:1024],
                                     start=True, stop=True, tile_position=(32, 0))
                    # two: out[p=(i,j), f=(k,l)] = sum_m at2[m,(i,j)] ab[m,(k,l)]
                    nc.tensor.matmul(tp[:, 0:512], att[64:96, ms], ab4[64:96, 0:512],
                                     start=True, stop=True, tile_position=(64, 0))
                    nc.tensor.matmul(tp[:, 512:1024], att[96:128, ms], ab4[96:128, 512:1024],
                                     start=True, stop=True, tile_position=(96, 0))

                    if pend is not None:
                        emit_h(h4, af, pend)

                    # Ln(tp) first: frees the single-buffered tp pool
                    # earlier.  Final chunk only: Ln(op) first so the last
                    # transpose overlaps the last activation (shorter tail).
                    lt = spool.tile([128, 1024], BF16, tag="lt")
                    lo = spool.tile([128, 1024], BF16, tag="lo")
                    if b == b_local - 1 and c == NCHUNK - 1:
                        nc.scalar.activation(lo[:], op[:], Ln)
                        nc.scalar.activation(lt[:], tp[:], Ln)
                    else:
                        nc.scalar.activation(lt[:], tp[:], Ln)
                        nc.scalar.activation(lo[:], op[:], Ln)

                    lot = spool.tile([128, 1024], BF16, tag="lot")
                    nc.vector.transpose(lot[:], lo[:])

                    pend = (lt, lot, cs, c == 0, c == NCHUNK - 1)

                emit_h(h4, af, pend)
                # drain: acc[:, b] = rowsum((abq*sgn) ⊙ H4)
                nc.vector.scalar_tensor_tensor(
                    out=junk[:], in0=abq[:], scalar=sgn[:, 0:1], in1=h4[:],
                    op0=mult, op1=mult, accum_out=acc[:, b:b + 1],
                )

            nc.sync.dma_start(out=out_ext[:, :], in_=acc[:])

    nc.compile()
    return nc


def kernel(cayley_cube: np.ndarray) -> np.ndarray:
    assert cayley_cube.shape == (B, N, N, N)
    nc = build()
    shards = cayley_cube.reshape(N_CORES, B_LOCAL, N, N, N)
    in_maps = [
        {"cayley_cube": np.ascontiguousarray(shards[i])} for i in range(N_CORES)
    ]
    res = run_bass_kernel_spmd(nc, in_maps, core_ids=list(range(N_CORES)))
    tot = np.float64(0.0)
    for r in res.results:
        tot += r["out"].sum(dtype=np.float64)
    return np.float32(tot / B)


if __name__ == "__main__":
    rng = np.random.default_rng(0)
    raw = rng.uniform(0.05, 1.0, size=(B, N, N, N)).astype(np.float32)
    a = raw / raw.sum(axis=-1, keepdims=True)
    print(kernel(a))


# revision 39
# speedup vs baseline: 1.0394x; 1.0394x over previous
"""Trainium2 Bass kernel for nn_AssociatorLoss (low-rank dot formulation).

Reference (B=32, N=32), a = cayley_cube (B,N,N,N):
    one[b,i,j,k,l] = sum_m a[b,i,m,l] * a[b,j,k,m]
    two[b,i,j,k,l] = sum_m a[b,m,k,l] * a[b,i,j,m]
    kl = sum(two * (log(two) - log(one))) / B

Key identity: in (u=(i,j), v=(k,l)) coordinates two = P·Q with
P[u,m] = a[i,j,m], Q[m,v] = a[m,k,l] (rank 32).  Hence for any X in that
layout:  sum(two ⊙ X) = sum_m,v Q[m,v] · (P^T X)[m,v].
So both dot products reduce to small PE matmuls H = P^T X accumulated over
(i,j)-chunks, with X = ln(two) and X = blockT(ln(one)):

per chunk c (128 (ij)-rows x 1024 (kl)-cols), per batch elem:
  PE : op = one-chunk  [p=(i,l), f=(k,j)]  (2 row-packed K=32 matmuls)
       tp = two-chunk  [p=(i,j), f=(k,l)]  (2 row-packed K=32 matmuls)
  ACT: lt = Ln(tp), lo = Ln(op)   (PSUM -> SBUF bf16)
  DVE: lot = blockT32(lo)         -> [p=(i,j), f=(k,l)]
  PE : H4 quadrants += af_c^T @ {lt,lot}-halves  (4 col-packed matmuls,
       K=128, M=32, accumulated over the 8 chunks in PSUM)
per batch elem:
  DVE: acc[:,b] = rowsum( (abq * sgn) ⊙ H4 )   (sgn=+1 for ln-two quads,
       -1 for ln-one quads); host sums acc and divides by B.

Data-parallel over b: 4 batch elems per core, partial sums combined on host.
"""

import sys

for _p in ("/opt/trn_rl_repo",):
    if _p not in sys.path:
        sys.path.insert(0, _p)

import numpy as np

import concourse.bacc as bacc
import concourse.mybir as mybir
import concourse.tile as tile
from concourse.bass_utils import run_bass_kernel_spmd

B, N = 32, 32
N_CORES = 8
B_LOCAL = B // N_CORES  # 4
NCHUNK = (N * N) // 128  # 8 chunks of 128 (ij)-rows per batch element
F32 = mybir.dt.float32
BF16 = mybir.dt.bfloat16


def build(b_local=B_LOCAL):
    nc = bacc.Bacc(None, target_bir_lowering=False)
    a_ext = nc.declare_dram_parameter("cayley_cube", [b_local, N, N, N], F32, isOutput=False)
    out_ext = nc.declare_dram_parameter("out", [128, b_local], F32, isOutput=True)

    av = a_ext.rearrange("b x y z -> b x (y z)")
    # af rows are (ij)-chunk layout: af[p, c*32+m] = a[4c + p//32, p%32, m]
    av4 = a_ext.rearrange("b (c il) j m -> b (il j) c m", c=NCHUNK, il=4)
    # abq halves: row h*32+m holds a[m, (kl) half h]
    av5 = a_ext.rearrange("b m (h k2) l -> b h m (k2 l)", h=2, k2=16)
    # aof[m, (c,il,l)] = a[4c+il, m, l] == ay2[:, 128c:...] (one-matmul lhsT)
    av6 = a_ext.rearrange("b (c il) m l -> b m c il l", c=NCHUNK, il=4)

    mult = mybir.AluOpType.mult
    Ln = mybir.ActivationFunctionType.Ln

    with tile.TileContext(nc) as tc:
        with (
            tc.tile_pool(name="apool", bufs=2) as apool,
            tc.tile_pool(name="spool", bufs=4) as spool,
            tc.tile_pool(name="scratch", bufs=1) as scratch,
            tc.tile_pool(name="psumO", bufs=2, space="PSUM") as psumO,
            tc.tile_pool(name="psumT", bufs=1, space="PSUM") as psumT,
            tc.tile_pool(name="psumH", bufs=2, space="PSUM") as psumH,
        ):
            sgn = scratch.tile([128, 1], F32)
            nc.vector.memset(sgn[0:64, :], 1.0)
            nc.vector.memset(sgn[64:128, :], -1.0)
            acc = scratch.tile([128, b_local], F32)
            junk = scratch.tile([128, 512], BF16)

            def emit_prep_dma(b):
                """Fire all HBM loads for batch b (casting loads on the
                gpsimd queue, plain f32 loads on sync)."""
                t = {}
                ab4 = apool.tile([128, 1024], BF16, tag="ab4")
                t["ab4"] = ab4
                nc.gpsimd.dma_start(out=ab4[0:32, :], in_=av[b])
                aof2 = apool.tile([64, NCHUNK * 128], BF16, tag="aof2")
                t["aof2"] = aof2
                for q in range(2):
                    nc.gpsimd.dma_start(
                        out=aof2[32 * q:32 * q + 32, :].rearrange(
                            "p (c il l) -> p c il l", c=NCHUNK, il=4, l=N),
                        in_=av6[b],
                    )
                for q in (2, 3):
                    nc.gpsimd.dma_start(out=ab4[32 * q:32 * q + 32, :], in_=av[b])
                af = apool.tile([128, NCHUNK * 32], BF16, tag="af")
                t["af"] = af
                nc.gpsimd.dma_start(
                    out=af[:].rearrange("p (c m) -> p c m", c=NCHUNK, m=N),
                    in_=av4[b],
                )
                abq = apool.tile([128, 512], F32, tag="abq")
                t["abq"] = abq
                for q in range(4):
                    nc.sync.dma_start(out=abq[32 * q:32 * q + 32, :],
                                      in_=av5[b, q % 2])
                return t

            def emit_prep_ops(t, first=False):
                """On-chip transposes for batch b.  at is built as two half
                transposes straight into the partitions each matmul reads,
                so no at replica copy is needed."""
                ab4 = t["ab4"]
                # att: [0:32, 0:512]=at-h1, [32:64, 512:]=at-h2, [64:96]=at2
                # at[z, y*32+x] = a[x,y,z]; at2[z, x*32+y] = a[x,y,z]
                att = apool.tile([128, 1024], BF16, tag="att")
                t["att"] = att
                nc.vector.transpose(att[0:32, 0:512], ab4[0:32, 0:512])
                nc.vector.transpose(att[32:64, 512:1024], ab4[0:32, 512:1024])
                # at2 from the two at halves (y<16 from h1, y>=16 from h2)
                if first:
                    # x-split: chunk 0 reads only cols 0:128, so emit those
                    # slices (both y-halves + replica) before the rest
                    for x0, x1 in ((0, 4), (4, N)):
                        nc.vector.tensor_copy(
                            att[64:96, 32 * x0:32 * x1].rearrange(
                                "p (x y) -> p y x", x=x1 - x0, y=N)[:, 0:16, :],
                            att[0:32, 0:512].rearrange(
                                "p (y x) -> p y x", y=16, x=N)[:, :, x0:x1],
                        )
                        nc.vector.tensor_copy(
                            att[64:96, 32 * x0:32 * x1].rearrange(
                                "p (x y) -> p y x", x=x1 - x0, y=N)[:, 16:32, :],
                            att[32:64, 512:1024].rearrange(
                                "p (y x) -> p y x", y=16, x=N)[:, :, x0:x1],
                        )
                        nc.vector.tensor_copy(att[96:128, 32 * x0:32 * x1],
                                              att[64:96, 32 * x0:32 * x1])
                else:
                    for pb in (64, 96):
                        nc.gpsimd.tensor_copy(
                            att[pb:pb + 32, :].rearrange(
                                "p (x y) -> p y x", x=N, y=N)[:, 0:16, :],
                            att[0:32, 0:512].rearrange("p (y x) -> p y x", y=16, x=N),
                        )
                        nc.gpsimd.tensor_copy(
                            att[pb:pb + 32, :].rearrange(
                                "p (x y) -> p y x", x=N, y=N)[:, 16:32, :],
                            att[32:64, 512:1024].rearrange("p (y x) -> p y x", y=16, x=N),
                        )

            def emit_h(h4, af, pend):
                """H-matmuls for a finished chunk (emitted one chunk late so
                the in-order PE queue never stalls on the DVE transpose)."""
                lt, lot, cs, st, sp = pend
                nc.tensor.matmul(h4[0:32, :], af[:, cs], lt[:, 0:512],
                                 start=st, stop=sp, tile_position=(0, 0))
                nc.tensor.matmul(h4[32:64, :], af[:, cs], lt[:, 512:1024],
                                 start=st, stop=sp, tile_position=(0, 32))
                nc.tensor.matmul(h4[64:96, :], af[:, cs], lot[:, 0:512],
                                 start=st, stop=sp, tile_position=(0, 64))
                nc.tensor.matmul(h4[96:128, :], af[:, cs], lot[:, 512:1024],
                                 start=st, stop=sp, tile_position=(0, 96))

            prep = emit_prep_dma(0)
            emit_prep_ops(prep, first=True)
            for b in range(b_local):
                ab4, att = prep["ab4"], prep["att"]
                aof2, af, abq = prep["aof2"], prep["af"], prep["abq"]
                h4 = psumH.tile([128, 512], F32, tag="h4")
                pend = None

                for c in range(NCHUNK):
                    if c == 0 and b + 1 < b_local:
                        nprep = emit_prep_dma(b + 1)
                    if c == 3 and b + 1 < b_local:
                        emit_prep_ops(nprep)
                    if c == 6 and b + 1 < b_local:
                        prep = nprep
                    ms = slice(128 * c, 128 * (c + 1))
                    cs = slice(32 * c, 32 * (c + 1))
                    op = psumO.tile([128, 1024], F32, tag="op")
                    tp = psumT.tile([128, 1024], F32, tag="tp")

                    # one: out[p=(i,l), f=(k,j)] = sum_m aof[m,(i,l)] at[m,(k,j)]
                    nc.tensor.matmul(op[:, 0:512], aof2[0:32, ms], att[0:32, 0:512],
                                     start=True, stop=True, tile_position=(0, 0))
                    nc.tensor.matmul(op[:, 512:1024], aof2[32:64, ms], att[32:64, 512:1024],
                                     start=True, stop=True, tile_position=(32, 0))
                    # two: out[p=(i,j), f=(k,l)] = sum_m at2[m,(i,j)] ab[m,(k,l)]
                    nc.tensor.matmul(tp[:, 0:512], att[64:96, ms], ab4[64:96, 0:512],
                                     start=True, stop=True, tile_position=(64, 0))
                    nc.tensor.matmul(tp[:, 512:1024], att[96:128, ms], ab4[96:128, 512:1024],
                                     start=True, stop=True, tile_position=(96, 0))

                    if pend is not None:
                        emit_h(h4, af, pend)

                    # Ln(tp) first: frees the single-buffered tp pool
                    # earlier.  Final chunk only: Ln(op) first so the last
                    # transpose overlaps the last activation (shorter tail).
                    lt = spool.tile([128, 1024], BF16, tag="lt")
                    lo = spool.tile([128, 1024], BF16, tag="lo")
                    if b == b_local - 1 and c == NCHUNK - 1:
                        nc.scalar.activation(lo[:], op[:], Ln)
                        nc.scalar.activation(lt[:], tp[:], Ln)
                    else:
                        nc.scalar.activation(lt[:], tp[:], Ln)
                        nc.scalar.activation(lo[:], op[:], Ln)

                    lot = spool.tile([128, 1024], BF16, tag="lot")
                    nc.vector.transpose(lot[:], lo[:])

                    pend = (lt, lot, cs, c == 0, c == NCHUNK - 1)

                emit_h(h4, af, pend)
                # drain: acc[:, b] = rowsum((abq*sgn) ⊙ H4)
                nc.vector.scalar_tensor_tensor(
                    out=junk[:], in0=abq[:], scalar=sgn[:, 0:1], in1=h4[:],
                    op0=mult, op1=mult, accum_out=acc[:, b:b + 1],
                )

            nc.sync.dma_start(out=out_ext[:, :], in_=acc[:])

    nc.compile()
    return nc


def kernel(cayley_cube: np.ndarray) -> np.ndarray:
    assert cayley_cube.shape == (B, N, N, N)
    nc = build()
    shards = cayley_cube.reshape(N_CORES, B_LOCAL, N, N, N)
    in_maps = [
        {"cayley_cube": np.ascontiguousarray(shards[i])} for i in range(N_CORES)
    ]
    res = run_bass_kernel_spmd(nc, in_maps, core_ids=list(range(N_CORES)))
    tot = np.float64(0.0)
    for r in res.results:
        tot += r["out"].sum(dtype=np.float64)
    return np.float32(tot / B)


if __name__ == "__main__":
    rng = np.random.default_rng(0)
    raw = rng.uniform(0.05, 1.0, size=(B, N, N, N)).astype(np.float32)
    a = raw / raw.sum(axis=-1, keepdims=True)
    print(kernel(a))
